# revision 17
# baseline (speedup 1.0000x reference)
"""LIF spike-train kernel for 8 TRN2 NeuronCores.

Reference semantics (per element over t = 0..15):
    u_t = u_{t-1} - o_{t-1} + x_t ;  o_t = (u_t > 1)

Layout: x is [b=32, c=128, h=32, w=32, t=16] f32; t is contiguous, so each
(b,c,h,w) element is an independent 16-float row. Batch is sharded across
the 8 cores (524288 rows/core), viewed as [128 partitions, 4096 rows, 16].

Scan formulation (bit-exact vs the reference):
    m_t = o_t - u_t  (negated post-spike potential; m_{-1} = 0)
    u_t = x_t - m_{t-1}     (tensor_sub, in place over x)
    m_t = (u_t > 1) - u_t   (scalar_tensor_tensor is_gt/subtract, exact)

Output compression: spikes are 0/1, so the device emits uint8 (4x less
store traffic; per-core DMA drops from ~187us to ~117us roofline). One
ScalarE pass per tile: s = Sign(u - 1) in {-1,0,+1} cast to u8; the host
decodes with (v == 1), exact whether the float->u8 cast saturates or wraps.

Engine split per tile: rows [0, solo) are scanned entirely by DVE; rows
[solo, R) ping-pong between the Pool/GpSimd engine (u = x - m tensor_sub,
the only scan op its ISA supports - comparisons are rejected) and DVE
(threshold STT), in 2 groups so the cross-engine semaphore latency of one
group hides behind the other group + solo work. The last tile's spikes are
extracted by DVE itself (tensor_scalar is_gt -> u8) since it is idle then.

Tile schedule: small head tiles cut time-to-first-compute; the act pass is
chunked (act_splits) and software-pipelined (store of chunk k issues after
activation k+1) so ScalarE never waits on HWDGE issue.

Cost-model timeline: ~141us/core vs ~196us for the f32-out DVE-only
baseline; DMA busy ~117us, DVE ~121us, Pool ~87us, ScalarE ~54us.
Validated bit-exact (0/67M mismatches) on 8 axon TRN2 cores.

HW races the cost model does NOT show (both observed on silicon):
- DVE RAW: consecutive DVE instructions do not interlock on SBUF
  write->read; instruction durations here (>=200ns) cover the ~60ns
  write-back latency of the previous op before its output is re-read.
- Pool(Q7) write visibility: another engine reading a Pool-written plane
  immediately after Pool's then_inc sees stale data nondeterministically
  (t=15 spike flips). Hence DVE computes the final t=15 subtract for ALL
  rows; Pool output is only ever consumed by DVE via the sP handshake,
  one instruction-chain later.
"""

import numpy as np

import concourse.bass as bass
import concourse.mybir as mybir
from concourse.bass_utils import run_bass_kernel_spmd

B, C, H, W, T = 32, 128, 32, 32, 16
N_CORES = 8
P = 128
ROWS_PER_PART = (B // N_CORES) * C * H * W // P  # 4096
F32 = mybir.dt.float32
U8 = mybir.dt.uint8

_cache = {}

TILES = (256, 480, 736, 736, 736, 736, 416)
SOLO_FRAC = 0.44
GROUPS = 2
NBUF = 4
NOBUF = 5
ACT_SPLITS = 4
RBUF = 736


def _build_nc(
    tiles=TILES,
    solo_frac=SOLO_FRAC,
    groups=GROUPS,
    nbuf=NBUF,
    nobuf=NOBUF,
    act_splits=ACT_SPLITS,
    rbuf=RBUF,
    dve_last=True,
):
    assert sum(tiles) == ROWS_PER_PART
    assert max(tiles) <= rbuf
    assert min(tiles) >= 64 * groups
    nc = bass.Bass()
    negone = nc.alloc_sbuf_tensor("const-neg1", [P, 1], F32)
    nc.gpsimd.memset(negone.ap(), -1.0)
    nc.const_aps.aps[(F32, -1.0)] = negone.ap()
    nc.all_engine_barrier()

    x_d = nc.declare_dram_parameter("x", [P, ROWS_PER_PART, T], F32, isOutput=False)
    o_d = nc.declare_dram_parameter("out", [P, ROWS_PER_PART, T], U8, isOutput=True)

    gt = mybir.AluOpType.is_gt
    sub = mybir.AluOpType.subtract
    Sign = mybir.ActivationFunctionType.Sign

    NT = len(tiles)
    offs = [sum(tiles[:i]) for i in range(NT)]

    def split(rows):
        solo = max(1, int(rows * solo_frac))
        pp = rows - solo
        bounds = []
        base = solo
        for g in range(groups):
            n = pp // groups + (1 if g < pp % groups else 0)
            bounds.append((base, base + n))
            base += n
        assert base == rows
        return solo, bounds

    def act_chunks(rows):
        n = (rows + act_splits - 1) // act_splits
        out = []
        a = 0
        while a < rows:
            b = min(rows, a + n)
            out.append((a, b))
            a = b
        return out

    max_solo = max(split(r)[0] for r in tiles)
    max_grp = max(b - a for r in tiles for (a, b) in split(r)[1])
    max_chunk = max(b - a for r in tiles for (a, b) in act_chunks(r))

    xb = [nc.alloc_sbuf_tensor(f"xb{i}", [P, rbuf, T], F32) for i in range(nbuf)]
    ob = [nc.alloc_sbuf_tensor(f"ob{i}", [P, max_chunk, T], U8) for i in range(nobuf)]
    mS = nc.alloc_sbuf_tensor("mS", [P, max_solo], F32)
    mG = [nc.alloc_sbuf_tensor(f"mG{g}", [P, max_grp], F32) for g in range(groups)]
    obD = (
        nc.alloc_sbuf_tensor("obD", [P, tiles[-1], T], U8) if dve_last else None
    )

    chunks_per_tile = [len(act_chunks(r)) for r in tiles]
    chunk_tot = [sum(chunks_per_tile[:i]) for i in range(NT + 1)]

    with (
        nc.Block() as block,
        nc.semaphore("sL") as sL,
        nc.semaphore("sS") as sS,
        nc.semaphore("sV") as sV,
        nc.semaphore("sW") as sW,
        nc.semaphore("sA") as sA,
        nc.semaphore("sD") as sD,
        nc.semaphore("sP") as sP,
        nc.semaphore("sE") as sE,
    ):
        @block.sync
        def _(sync):
            for j in range(NT):
                if j >= nbuf:
                    sync.wait_ge(sA, chunk_tot[j - nbuf + 1])
                sync.dma_start(
                    out=xb[j % nbuf].ap()[:, 0:tiles[j], :],
                    in_=x_d[:, offs[j]:offs[j] + tiles[j], :],
                ).then_inc(sL, 16)
            sync.wait_ge(sS, 16 * chunk_tot[NT])

        @block.vector
        def _(vec):
            pcnt = 0
            for j in range(NT):
                vec.wait_ge(sL, 16 * (j + 1))
                solo, bounds = split(tiles[j])
                xt = xb[j % nbuf].ap()
                ms = mS.ap()[:, 0:solo]
                for g in range(groups):
                    a, b = bounds[g]
                    x0 = xt[:, a:b, 0]
                    vec.scalar_tensor_tensor(
                        out=mG[g].ap()[:, 0:b - a], in0=x0, scalar=1.0,
                        in1=x0, op0=gt, op1=sub,
                    ).then_inc(sD, 1)
                x0 = xt[:, 0:solo, 0]
                vec.scalar_tensor_tensor(
                    out=ms, in0=x0, scalar=1.0, in1=x0, op0=gt, op1=sub
                )
                for t in range(1, T):
                    u_t = xt[:, 0:solo, t]
                    ins = vec.tensor_sub(out=u_t, in0=u_t, in1=ms)
                    if t == T - 1:
                        # DVE also computes u_15 for the pool rows: every
                        # value ScalarE later reads must be LAST WRITTEN BY
                        # DVE - on real HW, Pool(Q7) writes are not reliably
                        # visible to other engines right after its semaphore
                        # fires (t=15 flips observed on silicon; the cost
                        # model does not model this).
                        ins.then_inc(sV, 1)
                        for g in range(groups):
                            a, b = bounds[g]
                            u_g = xt[:, a:b, t]
                            vec.tensor_sub(
                                out=u_g, in0=u_g, in1=mG[g].ap()[:, 0:b - a]
                            ).then_inc(sV, 1)
                    else:
                        vec.scalar_tensor_tensor(
                            out=ms, in0=u_t, scalar=1.0, in1=u_t,
                            op0=gt, op1=sub,
                        )
                        for g in range(groups):
                            a, b = bounds[g]
                            pcnt += 1
                            vec.wait_ge(sP, pcnt)
                            u_g = xt[:, a:b, t]
                            vec.scalar_tensor_tensor(
                                out=mG[g].ap()[:, 0:b - a], in0=u_g,
                                scalar=1.0, in1=u_g, op0=gt, op1=sub,
                            ).then_inc(sD, 1)
            if dve_last:
                # last tile: DVE extracts spikes itself (it is idle now);
                # its own t=15 subs precede this in program order.
                xt = xb[(NT - 1) % nbuf].ap()
                for k, (a, b) in enumerate(act_chunks(tiles[NT - 1])):
                    vec.tensor_scalar(
                        out=obD.ap()[:, a:b, :], in0=xt[:, a:b, :],
                        scalar1=1.0, scalar2=None, op0=gt,
                    ).then_inc(sE, 1)

        @block.gpsimd
        def _(g_eng):
            dcnt = 0
            for j in range(NT):
                g_eng.wait_ge(sL, 16 * (j + 1))
                solo, bounds = split(tiles[j])
                xt = xb[j % nbuf].ap()
                base_d = dcnt
                for t in range(1, T - 1):
                    for g in range(groups):
                        a, b = bounds[g]
                        g_eng.wait_ge(sD, base_d + (t - 1) * groups + g + 1)
                        u_g = xt[:, a:b, t]
                        g_eng.tensor_sub(
                            out=u_g, in0=u_g, in1=mG[g].ap()[:, 0:b - a]
                        ).then_inc(sP, 1)
                dcnt += 15 * groups

        @block.scalar
        def _(sca):
            # software-pipelined: store of chunk k issues after act k+1
            pending = None  # (ci, ot, dram_lo, dram_hi)
            ci = 0
            n_act_tiles = NT - 1 if dve_last else NT
            for j in range(n_act_tiles):
                sca.wait_ge(sV, (1 + groups) * (j + 1))
                xt = xb[j % nbuf].ap()
                for (a, b) in act_chunks(tiles[j]):
                    if ci >= nobuf:
                        sca.wait_ge(sS, 16 * (ci - nobuf + 1))
                    ot = ob[ci % nobuf].ap()[:, 0:b - a, :]
                    sca.activation(
                        out=ot, in_=xt[:, a:b, :], func=Sign, bias=-1.0, scale=1.0
                    ).then_inc(sA, 1)
                    if pending is not None:
                        pci, pot, lo, hi = pending
                        sca.wait_ge(sA, pci + 1)
                        sca.dma_start(out=o_d[:, lo:hi, :], in_=pot).then_inc(sS, 16)
                    pending = (ci, ot, offs[j] + a, offs[j] + b)
                    ci += 1
            pci, pot, lo, hi = pending
            sca.wait_ge(sA, pci + 1)
            sca.dma_start(out=o_d[:, lo:hi, :], in_=pot).then_inc(sS, 16)
            if dve_last:
                jl = NT - 1
                for k, (a, b) in enumerate(act_chunks(tiles[jl])):
                    sca.wait_ge(sE, k + 1)
                    sca.dma_start(
                        out=o_d[:, offs[jl] + a:offs[jl] + b, :],
                        in_=obD.ap()[:, a:b, :],
                    ).then_inc(sS, 16)
    return nc


def _get_nc():
    if "nc" not in _cache:
        _cache["nc"] = _build_nc()
    return _cache["nc"]


def _run(x: np.ndarray, **spmd_kwargs):
    assert x.shape == (B, C, H, W, T), x.shape
    x = np.ascontiguousarray(x, dtype=np.float32)
    bpc = B // N_CORES
    in_maps = [
        {"x": x[k * bpc:(k + 1) * bpc].reshape(P, ROWS_PER_PART, T)}
        for k in range(N_CORES)
    ]
    res = run_bass_kernel_spmd(
        _get_nc(), in_maps, core_ids=list(range(N_CORES)), **spmd_kwargs
    )
    out = np.concatenate(
        [
            (res.results[k]["out"] == 1).astype(np.float32).reshape(bpc, C, H, W, T)
            for k in range(N_CORES)
        ],
        axis=0,
    )
    return out, res


def kernel(x: np.ndarray) -> np.ndarray:
    out, _ = _run(x)
    return out


def kernel_profiled(x: np.ndarray):
    try:
        out, res = _run(x, trace=True)
    except ModuleNotFoundError:
        out, res = _run(x)
    return out, res
